# revision 12
# baseline (speedup 1.0000x reference)
"""Chamfer-style Gaussian-splat matching loss on 8 Trainium2 NeuronCores.

Sharding (data-parallel over queries M): core c handles batch c//4, query
slice c%4 (2048 queries) against the full input cloud (8192) of its batch.

Device pipeline per core:
  - negsq[m,n] = 2a.b - |a|^2 - |b|^2 via K=13 float32r hi/lo-split matmuls
    (fp32-accurate), 4 concurrent per PSUM tile via PE row-tiling.
  - row side, 3-stage software pipeline over query tiles t (A(t+1) and
    B(t)/C(t-1) interleave so no engine queue blocks across tiles):
      A: matmuls + 4 strided [128,16,128] PSUM max-reduces -> exact
         64-entry chunk table; argmax chunk via reduce-max + fused
         (table==max)*iota sum-accumulate; indirect-DMA gather of the
         128-wide chunk coordinate window W (fp32).
      B: 3 fused scalar_tensor_tensor FMAs (DVE) re-derive the window
         scores; argmin position via reduce-max + (negw==max)*iota
         accumulate; indirect attribute gather.
      C: matched-attribute losses: one |diff| ScalarE Abs-accumulate over
         host-prescaled f16 attributes + rot |dot| terms.
  - col side: transposed-orientation matmuls; one [128,2048] PSUM
    max-reduce per input tile.
Host: cross-shard min-reduce for min_in_to_out, sums for scalar loss terms.
"""
import numpy as np

B, N, M = 2, 8192, 8192
NCORES = 8
SHARDS = 4
MLOC = M // SHARDS       # 2048
NMT = MLOC // 128        # 16
CW = 128                 # argmin chunk width
NCHK = N // CW           # 64 chunks per query row
NT = N // 128            # 64
NMC = MLOC // 512        # 4
DA = 56

POS_W, ROT_W, SCALE_W, OPAC_W, SH_W = 1.0, 0.5, 0.5, 0.3, 0.2

_cache = {}


def _rn12(x):
    u = np.ascontiguousarray(x.astype(np.float32)).view(np.uint32)
    lsb = (u >> np.uint32(12)) & np.uint32(1)
    rounded = u + np.uint32(0x7FF) + lsb
    return (rounded & np.uint32(0xFFFFF000)).view(np.float32)


def _build_program(reps=1):
    from contextlib import ExitStack
    import concourse.bass as bass
    import concourse.bacc as bacc
    import concourse.tile as tile
    from concourse import mybir

    F32 = mybir.dt.float32
    F16 = mybir.dt.float16
    F32R = mybir.dt.float32r
    U32 = mybir.dt.uint32
    AX = mybir.AxisListType.X
    MAX = mybir.AluOpType.max
    ADD = mybir.AluOpType.add
    MUL = mybir.AluOpType.mult
    SUB = mybir.AluOpType.subtract
    EQ = mybir.AluOpType.is_equal
    MIN = mybir.AluOpType.min
    Abs = mybir.ActivationFunctionType.Abs

    nc = bacc.Bacc("TRN2", target_bir_lowering=False, debug=False)

    ab_d = nc.dram_tensor("ab", [13, MLOC + N], F32R, kind="ExternalInput").ap()
    w_d = nc.dram_tensor("w", [NCHK, 4 * CW], F32, kind="ExternalInput").ap()
    asml_d = nc.dram_tensor("asml", [128, 4 * NMT], F32, kind="ExternalInput").ap()
    iota_d = nc.dram_tensor("iota", [128, 512], F32, kind="ExternalInput").ap()
    inattr_d = nc.dram_tensor("in_attr", [N, DA], F16, kind="ExternalInput").ap()
    outattr_d = nc.dram_tensor("out_attr", [128, DA * NMT], F16, kind="ExternalInput").ap()
    # out_all: 0:16 rowmax | 16:80 colmax | 80:96 attrsum | 96:112 rotabs
    out_d = nc.dram_tensor("out_all", [128, 112], F32, kind="ExternalOutput").ap()

    with tile.TileContext(nc) as tc:
        with ExitStack() as ctx:
            const_pool = ctx.enter_context(tc.tile_pool(name="const", bufs=1))
            tab_pool = ctx.enter_context(tc.tile_pool(name="tab", bufs=3))
            small_pool = ctx.enter_context(tc.tile_pool(name="small", bufs=5))
            wnd_pool = ctx.enter_context(tc.tile_pool(name="wnd", bufs=3))
            scr_pool = ctx.enter_context(tc.tile_pool(name="scr", bufs=3))
            psum_pool = ctx.enter_context(tc.tile_pool(name="psum", bufs=2, space="PSUM"))

            ab_s = const_pool.tile([128, MLOC + N], F32R)
            nc.sync.dma_start(ab_s[0:13, :], ab_d[:])
            for _g in range(1, 4):
                nc.sync.dma_start(ab_s[32 * _g:32 * _g + 13, :], ab_d[:])
            asml_s = const_pool.tile([128, 4 * NMT], F32)
            nc.sync.dma_start(asml_s[:], asml_d[:])
            iota_s = const_pool.tile([128, 512], F32)
            nc.sync.dma_start(iota_s[:], iota_d[:])
            outattr_s = const_pool.tile([128, DA * NMT], F16)
            nc.sync.dma_start(outattr_s[:], outattr_d[:])

            out_all = const_pool.tile([128, 112], F32)

            def stage_a(t):
                table_s = tab_pool.tile([128, NCHK], F32, tag="table")
                for q in range(4):
                    ps = psum_pool.tile([128, 2048], F32, tag="ps")
                    for j in range(4):
                        c = q * 4 + j
                        gb = 32 * j
                        nc.tensor.matmul(
                            ps[:, j * 512:(j + 1) * 512],
                            ab_s[gb:gb + 13, t * 128:(t + 1) * 128],
                            ab_s[gb:gb + 13, MLOC + c * 512:MLOC + (c + 1) * 512],
                            start=True, stop=True, tile_position=(gb, 0))
                    nc.vector.tensor_reduce(
                        table_s[:, q * 16:(q + 1) * 16],
                        ps[:].rearrange("p (j f) -> p j f", j=16),
                        axis=AX, op=MAX)

                tmax = small_pool.tile([128, 1], F32, tag="tmax")
                nc.vector.tensor_reduce(tmax[:], table_s[:], axis=AX, op=MAX)
                nc.scalar.copy(out_all[:, t:t + 1], tmax[:])
                eqt = small_pool.tile([128, NCHK], F32, tag="eqt")
                cif = small_pool.tile([128, 1], F32, tag="cif")
                nc.vector.scalar_tensor_tensor(
                    out=eqt[:], in0=table_s[:], scalar=tmax[:, 0:1],
                    in1=iota_s[:, 0:NCHK], op0=EQ, op1=MUL,
                    accum_out=cif[:])
                ci = small_pool.tile([128, 1], U32, tag="ci")
                nc.vector.tensor_scalar(out=ci[:], in0=cif[:],
                                        scalar1=float(NCHK - 1), scalar2=None,
                                        op0=MIN)

                wnd = wnd_pool.tile([128, 4 * CW], F32, tag="wnd")
                nc.gpsimd.indirect_dma_start(
                    out=wnd[:], out_offset=None, in_=w_d[:],
                    in_offset=bass.IndirectOffsetOnAxis(ap=ci[:, 0:1], axis=0),
                )
                return {"cif": cif, "wnd": wnd}

            def stage_b(t, st):
                wnd = st["wnd"]
                a0 = asml_s[:, 4 * t + 0: 4 * t + 1]
                a1 = asml_s[:, 4 * t + 1: 4 * t + 2]
                a2 = asml_s[:, 4 * t + 2: 4 * t + 3]
                u1 = scr_pool.tile([128, CW], F32, tag="u1")
                nc.vector.scalar_tensor_tensor(
                    out=u1[:], in0=wnd[:, 0:CW], scalar=a0,
                    in1=wnd[:, 3 * CW:4 * CW], op0=MUL, op1=ADD)
                u2 = scr_pool.tile([128, CW], F32, tag="u2")
                nc.vector.scalar_tensor_tensor(
                    out=u2[:], in0=wnd[:, CW:2 * CW], scalar=a1,
                    in1=u1[:], op0=MUL, op1=ADD)
                negw = scr_pool.tile([128, CW], F32, tag="negw")
                nc.vector.scalar_tensor_tensor(
                    out=negw[:], in0=wnd[:, 2 * CW:3 * CW], scalar=a2,
                    in1=u2[:], op0=MUL, op1=ADD)

                wmax = small_pool.tile([128, 1], F32, tag="wmax")
                nc.vector.tensor_reduce(wmax[:], negw[:], axis=AX, op=MAX)
                eqw = scr_pool.tile([128, CW], F32, tag="eqw")
                wif = small_pool.tile([128, 1], F32, tag="wif")
                nc.vector.scalar_tensor_tensor(
                    out=eqw[:], in0=negw[:], scalar=wmax[:, 0:1],
                    in1=iota_s[:, 0:CW], op0=EQ, op1=MUL,
                    accum_out=wif[:])

                cif = st["cif"]
                mf = small_pool.tile([128, 1], F32, tag="mf")
                nc.vector.tensor_scalar(out=mf[:], in0=cif[:], scalar1=float(CW),
                                        scalar2=wif[:, 0:1],
                                        op0=MUL, op1=ADD)
                mi = small_pool.tile([128, 1], U32, tag="mi")
                nc.vector.tensor_scalar(out=mi[:], in0=mf[:],
                                        scalar1=float(N - 1), scalar2=None,
                                        op0=MIN)

                g = small_pool.tile([128, DA], F16, tag="g")
                nc.gpsimd.indirect_dma_start(
                    out=g[:], out_offset=None, in_=inattr_d[:],
                    in_offset=bass.IndirectOffsetOnAxis(ap=mi[:, 0:1], axis=0),
                )
                st["g"] = g

            def stage_c(t, st):
                g = st["g"]
                oat = outattr_s[:, DA * t: DA * (t + 1)]
                diff = small_pool.tile([128, DA - 4], F32, tag="diff")
                nc.vector.tensor_tensor(out=diff[:], in0=oat[:, 4:DA],
                                        in1=g[:, 4:DA], op=SUB)
                absd = small_pool.tile([128, DA - 4], F32, tag="absd")
                nc.scalar.activation(absd[:], diff[:], Abs,
                                     accum_out=out_all[:, 80 + t:81 + t])
                rotm = small_pool.tile([128, 4], F32, tag="rotm")
                nc.vector.tensor_tensor(out=rotm[:], in0=oat[:, 0:4],
                                        in1=g[:, 0:4], op=MUL)
                rotd = small_pool.tile([128, 1], F32, tag="rotd")
                nc.vector.tensor_reduce(rotd[:], rotm[:], axis=AX, op=ADD)
                nc.scalar.activation(out_all[:, 96 + t:97 + t], rotd[:], Abs)

            def _body():
                sts = {}
                for t in range(NMT + 2):
                    if t < NMT:
                        sts[t] = stage_a(t)
                    if 0 <= t - 1 < NMT:
                        stage_b(t - 1, sts[t - 1])
                    if 0 <= t - 2 < NMT:
                        stage_c(t - 2, sts[t - 2])
                        del sts[t - 2]

                # o2: per input tile, 4 matmuls into one 4-bank psum; one
                # full-tile PSUM max-reduce -> per-input colmax
                for nt in range(NT):
                    ps = psum_pool.tile([128, 2048], F32, tag="ps")
                    for mc in range(NMC):
                        gb = 32 * mc
                        nc.tensor.matmul(
                            ps[:, mc * 512:(mc + 1) * 512],
                            ab_s[gb:gb + 13, MLOC + nt * 128:MLOC + (nt + 1) * 128],
                            ab_s[gb:gb + 13, mc * 512:(mc + 1) * 512],
                            start=True, stop=True, tile_position=(gb, 0))
                    nc.vector.tensor_reduce(out_all[:, 16 + nt:17 + nt], ps[:],
                                            axis=AX, op=MAX)

            for _rep in range(reps):
                _body()

            nc.sync.dma_start(out_d[:], out_all[:])

    nc.compile()
    return nc


def _make_runner(nc, donate=True):
    """Build the jitted SPMD callable for a compiled program."""
    import jax
    from jax.sharding import Mesh, PartitionSpec
    from jax.experimental.shard_map import shard_map
    from concourse import mybir
    import concourse.bass2jax as b2j

    b2j.install_neuronx_cc_hook()

    partition_name = nc.partition_id_tensor.name if nc.partition_id_tensor else None
    in_names, out_names, out_avals, zero_outs = [], [], [], []
    for alloc in nc.m.functions[0].allocations:
        if not isinstance(alloc, mybir.MemoryLocationSet):
            continue
        name = alloc.memorylocations[0].name
        if alloc.kind == "ExternalInput":
            if name != partition_name:
                in_names.append(name)
        elif alloc.kind == "ExternalOutput":
            out_names.append(name)
            shape = tuple(alloc.tensor_shape)
            dtype = mybir.dt.np(alloc.dtype)
            out_avals.append(jax.core.ShapedArray(shape, dtype))
            zero_outs.append(np.zeros(shape, dtype))
    n_params = len(in_names)
    n_outs = len(out_avals)
    all_in_names = list(in_names) + list(out_names)
    if partition_name is not None:
        all_in_names.append(partition_name)

    def _body(*args):
        operands = list(args)
        if partition_name is not None:
            operands.append(b2j.partition_id_tensor())
        outs = b2j._bass_exec_p.bind(
            *operands,
            out_avals=tuple(out_avals),
            in_names=tuple(all_in_names),
            out_names=tuple(out_names),
            lowering_input_output_aliases=(),
            sim_require_finite=True,
            sim_require_nnan=True,
            nc=nc,
        )
        return tuple(outs)

    devices = jax.devices()[:NCORES]
    mesh = Mesh(np.asarray(devices), ("core",))
    in_specs = (PartitionSpec("core"),) * (n_params + n_outs)
    out_specs = (PartitionSpec("core"),) * n_outs
    kwargs = dict(keep_unused=True)
    if donate:
        kwargs["donate_argnums"] = tuple(range(n_params, n_params + n_outs))
    sharded = jax.jit(
        shard_map(_body, mesh=mesh, in_specs=in_specs, out_specs=out_specs,
                  check_rep=False), **kwargs)
    return sharded, in_names, out_names, out_avals, zero_outs


def _build_runner():
    nc = _build_program()
    sharded, in_names, out_names, out_avals, zero_outs = _make_runner(nc)

    def run(in_maps):
        concat_in = [
            np.concatenate([np.asarray(in_maps[c][name]) for c in range(NCORES)], axis=0)
            for name in in_names
        ]
        concat_zeros = [np.zeros((NCORES * z.shape[0], *z.shape[1:]), z.dtype)
                        for z in zero_outs]
        out_arrs = sharded(*concat_in, *concat_zeros)
        return [
            {name: np.asarray(out_arrs[i]).reshape(NCORES, *out_avals[i].shape)[c]
             for i, name in enumerate(out_names)}
            for c in range(NCORES)
        ]

    return run


# per-column L1 weights folded into the f16 attribute tensors on the host
def _attr_weights():
    w_in = np.ones(DA, np.float32)
    w_in[4:7] = SCALE_W / 3.0
    w_in[7] = OPAC_W
    w_in[8:11] = SH_W / 3.0
    w_in[11:56] = SH_W / 45.0
    w_out = w_in.copy()
    w_out[0:4] = ROT_W
    return w_in, w_out


def _prep_core_inputs(core, in_xyz, in_attr_cat, out_xyz, out_attr_cat):
    b = core // SHARDS
    s = core % SHARDS
    a_xyz = np.ascontiguousarray(out_xyz[b, s * MLOC:(s + 1) * MLOC]).astype(np.float32)
    b_xyz = np.ascontiguousarray(in_xyz[b]).astype(np.float32)

    twoa = (2.0 * a_xyz.astype(np.float64)).astype(np.float32)
    ah = _rn12(twoa)
    al = _rn12(twoa - ah)
    bb = b_xyz.astype(np.float32)
    bh = _rn12(bb)
    bl = _rn12(bb - bh)
    na = -(a_xyz.astype(np.float64) ** 2).sum(-1)
    nb = -(b_xyz.astype(np.float64) ** 2).sum(-1)
    nah = _rn12(na.astype(np.float32))
    nal = _rn12((na - nah.astype(np.float64)).astype(np.float32))
    nbh = _rn12(nb.astype(np.float32))
    nbl = _rn12((nb - nbh.astype(np.float64)).astype(np.float32))
    om = np.ones((MLOC,), np.float32)
    on = np.ones((N,), np.float32)
    A13 = np.stack([ah[:, 0], ah[:, 0], al[:, 0],
                    ah[:, 1], ah[:, 1], al[:, 1],
                    ah[:, 2], ah[:, 2], al[:, 2],
                    nah, nal, om, om], axis=0)
    B13 = np.stack([bh[:, 0], bl[:, 0], bh[:, 0],
                    bh[:, 1], bl[:, 1], bh[:, 1],
                    bh[:, 2], bl[:, 2], bh[:, 2],
                    on, on, nbh, nbl], axis=0)
    ab = np.ascontiguousarray(np.concatenate([A13, B13], axis=1))

    W = np.empty((NCHK, 4 * CW), np.float32)
    nbf = nb.astype(np.float32)
    for c in range(NCHK):
        sl = slice(c * CW, (c + 1) * CW)
        W[c, 0:CW] = 2.0 * b_xyz[sl, 0]
        W[c, CW:2 * CW] = 2.0 * b_xyz[sl, 1]
        W[c, 2 * CW:3 * CW] = 2.0 * b_xyz[sl, 2]
        W[c, 3 * CW:4 * CW] = nbf[sl]

    naf = na.astype(np.float32)
    asml = np.stack([a_xyz[:, 0], a_xyz[:, 1], a_xyz[:, 2], naf], axis=1)
    asml_tiled = np.ascontiguousarray(
        asml.reshape(NMT, 128, 4).transpose(1, 0, 2).reshape(128, NMT * 4))

    oa = out_attr_cat[b, s * MLOC:(s + 1) * MLOC]
    oa_tiled = np.ascontiguousarray(
        oa.reshape(NMT, 128, DA).transpose(1, 0, 2).reshape(128, NMT * DA))

    iota = np.broadcast_to(np.arange(512, dtype=np.float32), (128, 512))

    return {
        "ab": ab,
        "w": W,
        "asml": asml_tiled,
        "iota": np.ascontiguousarray(iota),
        "in_attr": np.ascontiguousarray(in_attr_cat[b]),
        "out_attr": oa_tiled,
    }


def kernel(in_xyz, in_rot, in_scale, in_opacity, in_sh_dc, in_sh_rest,
           out_xyz, out_rot, out_scale, out_opacity, out_sh_dc, out_sh_rest):
    if "run" not in _cache:
        _cache["run"] = _build_runner()
    run = _cache["run"]

    w_in, w_out = _attr_weights()
    in_attr_cat = (np.concatenate(
        [in_rot, in_scale, in_opacity, in_sh_dc, in_sh_rest], axis=2
    ) * w_in).astype(np.float16)
    out_attr_cat = (np.concatenate(
        [out_rot, out_scale, out_opacity, out_sh_dc, out_sh_rest], axis=2
    ) * w_out).astype(np.float16)

    in_maps = [
        _prep_core_inputs(c, in_xyz, in_attr_cat, out_xyz, out_attr_cat)
        for c in range(NCORES)
    ]
    # Retry once: a crashed prior tenant can leave a core transiently wedged
    # (NRT_EXEC_UNIT_UNRECOVERABLE); it recovers after one failed attempt.
    outs_all = None
    last_err = None
    for _attempt in range(3):
        try:
            outs_all = run(in_maps)
            break
        except Exception as e:  # noqa: BLE001
            last_err = e
            import sys
            print(f"kernel attempt {_attempt} failed: "
                  f"{type(e).__name__}: {str(e)[:200]}", file=sys.stderr)
            import time as _time
            _time.sleep(3.0)
    if outs_all is None:
        raise last_err
    outs = [o["out_all"] for o in outs_all]

    row_sum = attr_sum = rot_sum = col_sum = 0.0
    for b in range(B):
        cores = [outs[b * SHARDS + s] for s in range(SHARDS)]
        colmax = cores[0][:, 16:80].copy()
        for s in range(1, SHARDS):
            np.maximum(colmax, cores[s][:, 16:80], out=colmax)
        col_sum += np.sqrt(np.maximum(-colmax, 0.0)).sum()
        for s in range(SHARDS):
            o = cores[s]
            row_sum += np.sqrt(np.maximum(-o[:, 0:16], 0.0)).sum()
            attr_sum += o[:, 80:96].sum()
            rot_sum += o[:, 96:112].sum()

    BM = B * M
    BN = B * N
    pos_loss = (row_sum / BM + col_sum / BN) / 2.0
    total = POS_W * pos_loss + ROT_W - rot_sum / BM + attr_sum / BM
    return np.float32(total)


# revision 13
# speedup vs baseline: 10.9938x; 10.9938x over previous
"""Chamfer-style Gaussian-splat matching loss on 8 Trainium2 NeuronCores.

Sharding (data-parallel over queries M): core c handles batch c//4, query
slice c%4 (2048 queries) against the full input cloud (8192) of its batch.

Device pipeline per core:
  - negsq[m,n] = 2a.b - |a|^2 - |b|^2 via K=13 float32r hi/lo-split matmuls
    (fp32-accurate), 4 concurrent per PSUM tile via PE row-tiling.
  - row side, 3-stage software pipeline over query tiles t (A(t+1) and
    B(t)/C(t-1) interleave so no engine queue blocks across tiles):
      A: matmuls + 4 strided [128,16,128] PSUM max-reduces -> exact
         64-entry chunk table; argmax chunk via reduce-max + fused
         (table==max)*iota sum-accumulate; indirect-DMA gather of the
         128-wide chunk coordinate window W (fp32).
      B: 3 fused scalar_tensor_tensor FMAs (DVE) re-derive the window
         scores; argmin position via reduce-max + (negw==max)*iota
         accumulate; indirect attribute gather.
      C: matched-attribute losses: one |diff| ScalarE Abs-accumulate over
         host-prescaled f16 attributes + rot |dot| terms.
  - col side: transposed-orientation matmuls; one [128,2048] PSUM
    max-reduce per input tile.
Host: cross-shard min-reduce for min_in_to_out, sums for scalar loss terms.
"""
import numpy as np

B, N, M = 2, 8192, 8192
NCORES = 8
SHARDS = 4
MLOC = M // SHARDS       # 2048
NMT = MLOC // 128        # 16
CW = 128                 # argmin chunk width
NCHK = N // CW           # 64 chunks per query row
NT = N // 128            # 64
NMC = MLOC // 512        # 4
DA = 56

POS_W, ROT_W, SCALE_W, OPAC_W, SH_W = 1.0, 0.5, 0.5, 0.3, 0.2

_cache = {}


def _rn12(x):
    u = np.ascontiguousarray(x.astype(np.float32)).view(np.uint32)
    lsb = (u >> np.uint32(12)) & np.uint32(1)
    rounded = u + np.uint32(0x7FF) + lsb
    return (rounded & np.uint32(0xFFFFF000)).view(np.float32)


def _build_program(reps=1, parts="all"):
    from contextlib import ExitStack
    import concourse.bass as bass
    import concourse.bacc as bacc
    import concourse.tile as tile
    from concourse import mybir

    F32 = mybir.dt.float32
    F16 = mybir.dt.float16
    F32R = mybir.dt.float32r
    U32 = mybir.dt.uint32
    AX = mybir.AxisListType.X
    MAX = mybir.AluOpType.max
    ADD = mybir.AluOpType.add
    MUL = mybir.AluOpType.mult
    SUB = mybir.AluOpType.subtract
    EQ = mybir.AluOpType.is_equal
    MIN = mybir.AluOpType.min
    Abs = mybir.ActivationFunctionType.Abs

    nc = bacc.Bacc("TRN2", target_bir_lowering=False, debug=False)

    ab_d = nc.dram_tensor("ab", [13, MLOC + N], F32R, kind="ExternalInput").ap()
    w_d = nc.dram_tensor("w", [NCHK, 4 * CW], F32, kind="ExternalInput").ap()
    asml_d = nc.dram_tensor("asml", [128, 4 * NMT], F32, kind="ExternalInput").ap()
    iota_d = nc.dram_tensor("iota", [128, 512], F32, kind="ExternalInput").ap()
    inattr_d = nc.dram_tensor("in_attr", [N, DA], F16, kind="ExternalInput").ap()
    outattr_d = nc.dram_tensor("out_attr", [128, DA * NMT], F16, kind="ExternalInput").ap()
    # out_all: 0:16 rowmax | 16:80 colmax | 80:96 attrsum | 96:112 rotabs
    out_d = nc.dram_tensor("out_all", [128, 112], F32, kind="ExternalOutput").ap()

    with tile.TileContext(nc) as tc:
        with ExitStack() as ctx:
            const_pool = ctx.enter_context(tc.tile_pool(name="const", bufs=1))
            tab_pool = ctx.enter_context(tc.tile_pool(name="tab", bufs=3))
            small_pool = ctx.enter_context(tc.tile_pool(name="small", bufs=5))
            wnd_pool = ctx.enter_context(tc.tile_pool(name="wnd", bufs=3))
            scr_pool = ctx.enter_context(tc.tile_pool(name="scr", bufs=3))
            psum_pool = ctx.enter_context(tc.tile_pool(name="psum", bufs=2, space="PSUM"))

            ab_s = const_pool.tile([128, MLOC + N], F32R)
            nc.sync.dma_start(ab_s[0:13, :], ab_d[:])
            for _g in range(1, 4):
                nc.sync.dma_start(ab_s[32 * _g:32 * _g + 13, :], ab_d[:])
            asml_s = const_pool.tile([128, 4 * NMT], F32)
            nc.sync.dma_start(asml_s[:], asml_d[:])
            iota_s = const_pool.tile([128, 512], F32)
            nc.sync.dma_start(iota_s[:], iota_d[:])
            outattr_s = const_pool.tile([128, DA * NMT], F16)
            nc.sync.dma_start(outattr_s[:], outattr_d[:])

            out_all = const_pool.tile([128, 112], F32)

            def stage_a(t):
                table_s = tab_pool.tile([128, NCHK], F32, tag="table")
                for q in range(4):
                    ps = psum_pool.tile([128, 2048], F32, tag="ps")
                    for j in range(4):
                        c = q * 4 + j
                        gb = 32 * j
                        nc.tensor.matmul(
                            ps[:, j * 512:(j + 1) * 512],
                            ab_s[gb:gb + 13, t * 128:(t + 1) * 128],
                            ab_s[gb:gb + 13, MLOC + c * 512:MLOC + (c + 1) * 512],
                            start=True, stop=True, tile_position=(gb, 0))
                    nc.vector.tensor_reduce(
                        table_s[:, q * 16:(q + 1) * 16],
                        ps[:].rearrange("p (j f) -> p j f", j=16),
                        axis=AX, op=MAX)

                tmax = small_pool.tile([128, 1], F32, tag="tmax")
                nc.vector.tensor_reduce(tmax[:], table_s[:], axis=AX, op=MAX)
                nc.scalar.copy(out_all[:, t:t + 1], tmax[:])
                eqt = small_pool.tile([128, NCHK], F32, tag="eqt")
                cif = small_pool.tile([128, 1], F32, tag="cif")
                nc.vector.scalar_tensor_tensor(
                    out=eqt[:], in0=table_s[:], scalar=tmax[:, 0:1],
                    in1=iota_s[:, 0:NCHK], op0=EQ, op1=MUL,
                    accum_out=cif[:])
                ci = small_pool.tile([128, 1], U32, tag="ci")
                nc.vector.tensor_scalar(out=ci[:], in0=cif[:],
                                        scalar1=float(NCHK - 1), scalar2=None,
                                        op0=MIN)

                wnd = wnd_pool.tile([128, 4 * CW], F32, tag="wnd")
                nc.gpsimd.indirect_dma_start(
                    out=wnd[:], out_offset=None, in_=w_d[:],
                    in_offset=bass.IndirectOffsetOnAxis(ap=ci[:, 0:1], axis=0),
                )
                return {"cif": cif, "wnd": wnd}

            def stage_b(t, st):
                wnd = st["wnd"]
                a0 = asml_s[:, 4 * t + 0: 4 * t + 1]
                a1 = asml_s[:, 4 * t + 1: 4 * t + 2]
                a2 = asml_s[:, 4 * t + 2: 4 * t + 3]
                u1 = scr_pool.tile([128, CW], F32, tag="u1")
                nc.vector.scalar_tensor_tensor(
                    out=u1[:], in0=wnd[:, 0:CW], scalar=a0,
                    in1=wnd[:, 3 * CW:4 * CW], op0=MUL, op1=ADD)
                u2 = scr_pool.tile([128, CW], F32, tag="u2")
                nc.vector.scalar_tensor_tensor(
                    out=u2[:], in0=wnd[:, CW:2 * CW], scalar=a1,
                    in1=u1[:], op0=MUL, op1=ADD)
                negw = scr_pool.tile([128, CW], F32, tag="negw")
                nc.vector.scalar_tensor_tensor(
                    out=negw[:], in0=wnd[:, 2 * CW:3 * CW], scalar=a2,
                    in1=u2[:], op0=MUL, op1=ADD)

                wmax = small_pool.tile([128, 1], F32, tag="wmax")
                nc.vector.tensor_reduce(wmax[:], negw[:], axis=AX, op=MAX)
                eqw = scr_pool.tile([128, CW], F32, tag="eqw")
                wif = small_pool.tile([128, 1], F32, tag="wif")
                nc.vector.scalar_tensor_tensor(
                    out=eqw[:], in0=negw[:], scalar=wmax[:, 0:1],
                    in1=iota_s[:, 0:CW], op0=EQ, op1=MUL,
                    accum_out=wif[:])

                cif = st["cif"]
                mf = small_pool.tile([128, 1], F32, tag="mf")
                nc.vector.tensor_scalar(out=mf[:], in0=cif[:], scalar1=float(CW),
                                        scalar2=wif[:, 0:1],
                                        op0=MUL, op1=ADD)
                mi = small_pool.tile([128, 1], U32, tag="mi")
                nc.vector.tensor_scalar(out=mi[:], in0=mf[:],
                                        scalar1=float(N - 1), scalar2=None,
                                        op0=MIN)

                g = small_pool.tile([128, DA], F16, tag="g")
                nc.gpsimd.indirect_dma_start(
                    out=g[:], out_offset=None, in_=inattr_d[:],
                    in_offset=bass.IndirectOffsetOnAxis(ap=mi[:, 0:1], axis=0),
                )
                st["g"] = g

            def stage_c(t, st):
                g = st["g"]
                oat = outattr_s[:, DA * t: DA * (t + 1)]
                diff = small_pool.tile([128, DA - 4], F32, tag="diff")
                nc.vector.tensor_tensor(out=diff[:], in0=oat[:, 4:DA],
                                        in1=g[:, 4:DA], op=SUB)
                absd = small_pool.tile([128, DA - 4], F32, tag="absd")
                nc.scalar.activation(absd[:], diff[:], Abs,
                                     accum_out=out_all[:, 80 + t:81 + t])
                rotm = small_pool.tile([128, 4], F32, tag="rotm")
                nc.vector.tensor_tensor(out=rotm[:], in0=oat[:, 0:4],
                                        in1=g[:, 0:4], op=MUL)
                rotd = small_pool.tile([128, 1], F32, tag="rotd")
                nc.vector.tensor_reduce(rotd[:], rotm[:], axis=AX, op=ADD)
                nc.scalar.activation(out_all[:, 96 + t:97 + t], rotd[:], Abs)

            def _body():
                sts = {}
                if parts in ("all", "o1", "o1a"):
                    for t in range(NMT + 2):
                        if t < NMT:
                            sts[t] = stage_a(t)
                        if parts == "o1a":
                            continue
                        if 0 <= t - 1 < NMT:
                            stage_b(t - 1, sts[t - 1])
                        if 0 <= t - 2 < NMT:
                            stage_c(t - 2, sts[t - 2])
                            del sts[t - 2]

                # o2: per input tile, 4 matmuls into one 4-bank psum; one
                # full-tile PSUM max-reduce -> per-input colmax
                for nt in range(NT if parts in ("all", "o2") else 0):
                    ps = psum_pool.tile([128, 2048], F32, tag="ps")
                    for mc in range(NMC):
                        gb = 32 * mc
                        nc.tensor.matmul(
                            ps[:, mc * 512:(mc + 1) * 512],
                            ab_s[gb:gb + 13, MLOC + nt * 128:MLOC + (nt + 1) * 128],
                            ab_s[gb:gb + 13, mc * 512:(mc + 1) * 512],
                            start=True, stop=True, tile_position=(gb, 0))
                    nc.vector.tensor_reduce(out_all[:, 16 + nt:17 + nt], ps[:],
                                            axis=AX, op=MAX)

            for _rep in range(reps):
                _body()

            nc.sync.dma_start(out_d[:], out_all[:])

    nc.compile()
    return nc


def _make_runner(nc, donate=True):
    """Build the jitted SPMD callable for a compiled program."""
    import jax
    from jax.sharding import Mesh, PartitionSpec
    from jax.experimental.shard_map import shard_map
    from concourse import mybir
    import concourse.bass2jax as b2j

    b2j.install_neuronx_cc_hook()

    partition_name = nc.partition_id_tensor.name if nc.partition_id_tensor else None
    in_names, out_names, out_avals, zero_outs = [], [], [], []
    for alloc in nc.m.functions[0].allocations:
        if not isinstance(alloc, mybir.MemoryLocationSet):
            continue
        name = alloc.memorylocations[0].name
        if alloc.kind == "ExternalInput":
            if name != partition_name:
                in_names.append(name)
        elif alloc.kind == "ExternalOutput":
            out_names.append(name)
            shape = tuple(alloc.tensor_shape)
            dtype = mybir.dt.np(alloc.dtype)
            out_avals.append(jax.core.ShapedArray(shape, dtype))
            zero_outs.append(np.zeros(shape, dtype))
    n_params = len(in_names)
    n_outs = len(out_avals)
    all_in_names = list(in_names) + list(out_names)
    if partition_name is not None:
        all_in_names.append(partition_name)

    def _body(*args):
        operands = list(args)
        if partition_name is not None:
            operands.append(b2j.partition_id_tensor())
        outs = b2j._bass_exec_p.bind(
            *operands,
            out_avals=tuple(out_avals),
            in_names=tuple(all_in_names),
            out_names=tuple(out_names),
            lowering_input_output_aliases=(),
            sim_require_finite=True,
            sim_require_nnan=True,
            nc=nc,
        )
        return tuple(outs)

    devices = jax.devices()[:NCORES]
    mesh = Mesh(np.asarray(devices), ("core",))
    in_specs = (PartitionSpec("core"),) * (n_params + n_outs)
    out_specs = (PartitionSpec("core"),) * n_outs
    kwargs = dict(keep_unused=True)
    if donate:
        kwargs["donate_argnums"] = tuple(range(n_params, n_params + n_outs))
    sharded = jax.jit(
        shard_map(_body, mesh=mesh, in_specs=in_specs, out_specs=out_specs,
                  check_rep=False), **kwargs)
    return sharded, in_names, out_names, out_avals, zero_outs


def _build_runner():
    nc = _build_program()
    sharded, in_names, out_names, out_avals, zero_outs = _make_runner(nc)

    def run(in_maps):
        concat_in = [
            np.concatenate([np.asarray(in_maps[c][name]) for c in range(NCORES)], axis=0)
            for name in in_names
        ]
        concat_zeros = [np.zeros((NCORES * z.shape[0], *z.shape[1:]), z.dtype)
                        for z in zero_outs]
        out_arrs = sharded(*concat_in, *concat_zeros)
        return [
            {name: np.asarray(out_arrs[i]).reshape(NCORES, *out_avals[i].shape)[c]
             for i, name in enumerate(out_names)}
            for c in range(NCORES)
        ]

    return run


# per-column L1 weights folded into the f16 attribute tensors on the host
def _attr_weights():
    w_in = np.ones(DA, np.float32)
    w_in[4:7] = SCALE_W / 3.0
    w_in[7] = OPAC_W
    w_in[8:11] = SH_W / 3.0
    w_in[11:56] = SH_W / 45.0
    w_out = w_in.copy()
    w_out[0:4] = ROT_W
    return w_in, w_out


def _prep_core_inputs(core, in_xyz, in_attr_cat, out_xyz, out_attr_cat):
    b = core // SHARDS
    s = core % SHARDS
    a_xyz = np.ascontiguousarray(out_xyz[b, s * MLOC:(s + 1) * MLOC]).astype(np.float32)
    b_xyz = np.ascontiguousarray(in_xyz[b]).astype(np.float32)

    twoa = (2.0 * a_xyz.astype(np.float64)).astype(np.float32)
    ah = _rn12(twoa)
    al = _rn12(twoa - ah)
    bb = b_xyz.astype(np.float32)
    bh = _rn12(bb)
    bl = _rn12(bb - bh)
    na = -(a_xyz.astype(np.float64) ** 2).sum(-1)
    nb = -(b_xyz.astype(np.float64) ** 2).sum(-1)
    nah = _rn12(na.astype(np.float32))
    nal = _rn12((na - nah.astype(np.float64)).astype(np.float32))
    nbh = _rn12(nb.astype(np.float32))
    nbl = _rn12((nb - nbh.astype(np.float64)).astype(np.float32))
    om = np.ones((MLOC,), np.float32)
    on = np.ones((N,), np.float32)
    A13 = np.stack([ah[:, 0], ah[:, 0], al[:, 0],
                    ah[:, 1], ah[:, 1], al[:, 1],
                    ah[:, 2], ah[:, 2], al[:, 2],
                    nah, nal, om, om], axis=0)
    B13 = np.stack([bh[:, 0], bl[:, 0], bh[:, 0],
                    bh[:, 1], bl[:, 1], bh[:, 1],
                    bh[:, 2], bl[:, 2], bh[:, 2],
                    on, on, nbh, nbl], axis=0)
    ab = np.ascontiguousarray(np.concatenate([A13, B13], axis=1))

    W = np.empty((NCHK, 4 * CW), np.float32)
    nbf = nb.astype(np.float32)
    for c in range(NCHK):
        sl = slice(c * CW, (c + 1) * CW)
        W[c, 0:CW] = 2.0 * b_xyz[sl, 0]
        W[c, CW:2 * CW] = 2.0 * b_xyz[sl, 1]
        W[c, 2 * CW:3 * CW] = 2.0 * b_xyz[sl, 2]
        W[c, 3 * CW:4 * CW] = nbf[sl]

    naf = na.astype(np.float32)
    asml = np.stack([a_xyz[:, 0], a_xyz[:, 1], a_xyz[:, 2], naf], axis=1)
    asml_tiled = np.ascontiguousarray(
        asml.reshape(NMT, 128, 4).transpose(1, 0, 2).reshape(128, NMT * 4))

    oa = out_attr_cat[b, s * MLOC:(s + 1) * MLOC]
    oa_tiled = np.ascontiguousarray(
        oa.reshape(NMT, 128, DA).transpose(1, 0, 2).reshape(128, NMT * DA))

    iota = np.broadcast_to(np.arange(512, dtype=np.float32), (128, 512))

    return {
        "ab": ab,
        "w": W,
        "asml": asml_tiled,
        "iota": np.ascontiguousarray(iota),
        "in_attr": np.ascontiguousarray(in_attr_cat[b]),
        "out_attr": oa_tiled,
    }


def kernel(in_xyz, in_rot, in_scale, in_opacity, in_sh_dc, in_sh_rest,
           out_xyz, out_rot, out_scale, out_opacity, out_sh_dc, out_sh_rest):
    if "run" not in _cache:
        _cache["run"] = _build_runner()
    run = _cache["run"]

    w_in, w_out = _attr_weights()
    in_attr_cat = (np.concatenate(
        [in_rot, in_scale, in_opacity, in_sh_dc, in_sh_rest], axis=2
    ) * w_in).astype(np.float16)
    out_attr_cat = (np.concatenate(
        [out_rot, out_scale, out_opacity, out_sh_dc, out_sh_rest], axis=2
    ) * w_out).astype(np.float16)

    in_maps = [
        _prep_core_inputs(c, in_xyz, in_attr_cat, out_xyz, out_attr_cat)
        for c in range(NCORES)
    ]
    # Retry once: a crashed prior tenant can leave a core transiently wedged
    # (NRT_EXEC_UNIT_UNRECOVERABLE); it recovers after one failed attempt.
    outs_all = None
    last_err = None
    for _attempt in range(3):
        try:
            outs_all = run(in_maps)
            break
        except Exception as e:  # noqa: BLE001
            last_err = e
            import sys
            print(f"kernel attempt {_attempt} failed: "
                  f"{type(e).__name__}: {str(e)[:200]}", file=sys.stderr)
            import time as _time
            _time.sleep(3.0)
    if outs_all is None:
        raise last_err
    outs = [o["out_all"] for o in outs_all]

    row_sum = attr_sum = rot_sum = col_sum = 0.0
    for b in range(B):
        cores = [outs[b * SHARDS + s] for s in range(SHARDS)]
        colmax = cores[0][:, 16:80].copy()
        for s in range(1, SHARDS):
            np.maximum(colmax, cores[s][:, 16:80], out=colmax)
        col_sum += np.sqrt(np.maximum(-colmax, 0.0)).sum()
        for s in range(SHARDS):
            o = cores[s]
            row_sum += np.sqrt(np.maximum(-o[:, 0:16], 0.0)).sum()
            attr_sum += o[:, 80:96].sum()
            rot_sum += o[:, 96:112].sum()

    BM = B * M
    BN = B * N
    pos_loss = (row_sum / BM + col_sum / BN) / 2.0
    total = POS_W * pos_loss + ROT_W - rot_sum / BM + attr_sum / BM
    return np.float32(total)
